# revision 25
# baseline (speedup 1.0000x reference)
"""Trainium2 Bass kernel for the CoordPooling+SFP gate module (bf16).

Computation (per batch b):
  y_pre = [sum_w x | sum_h x]                 [C, H+W]   (C=384, H=W=64)
  y  = relu((sy*Wy/64 @ y_pre) + by)          [C, 128]
  xh = relu((sh*Wh @ y[:, :64]) + bh)         [C, 64]
  xw = relu((sw*Ww @ y[:, 64:]) + bw)         [C, 64]
  z_raw[c] = sum_L y[c, :]
  z  = fc1(relu(bn1(fc0(z_raw * wscale))))    [C]
  out = x * (sigmoid(xh outer xw) + z)

Everything on-chip is bf16 (HBM traffic halved; DVE 2x mode on packed
ops); accumulations happen in fp32 inside the engines.  Reductions are
TT fold trees (tensor_reduce gets no bf16 speedup).  The outer product
runs at DVE 2x via duplicated xh pairs: the hw-matmul streams each y
column twice so relu lands xh as [h,2] pairs, letting the broadcast sit
on a middle AP dim with a stride-1 innermost pair.

Sharding: data-parallel over batch, 4 batches per core on 8 cores.
"""

import sys
import numpy as np
import ml_dtypes

for _p in ("/opt/trn_rl_repo", "/root/.axon_site/_ro/trn_rl_repo"):
    if _p not in sys.path:
        sys.path.append(_p)

def _ensure_ntff_hook():
    # bass_utils imports antenv.axon_hooks when tracing is requested
    # (e.g. BASS_TRACE=1); this image's antenv lacks that module, which
    # would crash the run.  Register a functional shim (real ctypes hook
    # when available, else a None-hook so tracing degrades gracefully).
    try:
        import antenv.axon_hooks  # noqa: F401
        return
    except ImportError:
        pass
    import types
    try:
        import antenv
    except ImportError:
        return
    mod = types.ModuleType("antenv.axon_hooks")
    _hook = [None]
    mod.set_axon_ntff_profile_hook = lambda h: _hook.__setitem__(0, h)
    mod.get_axon_ntff_profile_hook = lambda: _hook[0]
    sys.modules["antenv.axon_hooks"] = mod
    antenv.axon_hooks = mod
    try:
        from trn_agent_boot.trn_boot import _ntff_profile_via_ctypes
        mod.set_axon_ntff_profile_hook(
            _ntff_profile_via_ctypes("/opt/axon/libaxon_pjrt.so"))
    except Exception:
        pass


_ensure_ntff_hook()

import concourse.bass as bass
import concourse.tile as tile
from concourse import bacc, mybir
from concourse.bass_utils import run_bass_kernel_spmd

F32 = mybir.dt.float32
BF16 = mybir.dt.bfloat16
AF = mybir.ActivationFunctionType
OP = mybir.AluOpType
BF = ml_dtypes.bfloat16

N_CORES = 8
B, C, H, W = 32, 384, 64, 64
BS = B // N_CORES          # batches per core
P = 128                    # partitions
KC = C // P                # channel chunks (3)
R = 24                     # gate bottleneck
EPS = 1e-5

# const blob layout (free-dim offsets within [128, CONST_F])
_OFF_WY = 0
_OFF_WH = _OFF_WY + KC * C      # 1152
_OFF_WW = _OFF_WH + KC * C
WBLOB_F = _OFF_WW + KC * C      # 3456 (bf16 blob)
_OFF_FC0 = 0
_OFF_BY = _OFF_FC0 + KC * R     # 72
_OFF_BH = _OFF_BY + KC
_OFF_BW = _OFF_BH + KC
_OFF_FC1B = _OFF_BW + KC
CONST_F = _OFF_FC1B + KC        # 84

_ZOFF_FC1 = 0
_ZOFF_S = KC * P                # 384
_ZOFF_B = _ZOFF_S + 1
ZCONST_F = _ZOFF_B + 1          # 386

_compiled = None


def _build():
    nc = bacc.Bacc("TRN2", target_bir_lowering=False, debug=False,
                   num_devices=N_CORES)
    x_d = nc.dram_tensor("x", [BS, C, H, W], BF16, kind="ExternalInput")
    wbl_d = nc.dram_tensor("wbl", [P, WBLOB_F], BF16, kind="ExternalInput")
    cst_d = nc.dram_tensor("cst", [P, CONST_F], F32, kind="ExternalInput")
    zcst_d = nc.dram_tensor("zcst", [R, ZCONST_F], F32, kind="ExternalInput")
    out_d = nc.dram_tensor("out", [BS, C, H, W], BF16, kind="ExternalOutput")

    with tile.TileContext(nc) as tc:
        with (
            nc.allow_low_precision("bf16 pipeline, tolerance 2e-2"),
            tc.tile_pool(name="consts", bufs=1) as consts,
            tc.tile_pool(name="xp", bufs=4) as xpool,
            tc.tile_pool(name="wtp", bufs=2) as wtpool,
            tc.tile_pool(name="hfp", bufs=2) as hfpool,
            tc.tile_pool(name="ypre", bufs=2) as ypre_pool,
            tc.tile_pool(name="ysb", bufs=2) as ysb_pool,
            tc.tile_pool(name="hwp", bufs=2) as hw_pool,
            tc.tile_pool(name="zp", bufs=2) as zpool,
            tc.tile_pool(name="tp", bufs=8) as tpool,
            tc.tile_pool(name="psy", bufs=2, space=bass.MemorySpace.PSUM) as psy,
            tc.tile_pool(name="pshw", bufs=2, space=bass.MemorySpace.PSUM) as pshw,
            tc.tile_pool(name="psz", bufs=1, space=bass.MemorySpace.PSUM) as psz,
        ):
            wbl = consts.tile([P, WBLOB_F], BF16)
            nc.scalar.dma_start(wbl[:], wbl_d.ap())
            cst = consts.tile([P, CONST_F], F32)
            nc.scalar.dma_start(cst[:], cst_d.ap())
            zcst = consts.tile([R, ZCONST_F], F32)
            nc.scalar.dma_start(zcst[:], zcst_d.ap())

            wyT = wbl[:, _OFF_WY:_OFF_WH].rearrange("p (k o) -> p k o", k=KC)
            whT = wbl[:, _OFF_WH:_OFF_WW].rearrange("p (k o) -> p k o", k=KC)
            wwT = wbl[:, _OFF_WW:WBLOB_F].rearrange("p (k o) -> p k o", k=KC)
            fc0T = cst[:, _OFF_FC0:_OFF_BY].rearrange("p (k r) -> p k r", k=KC)
            by_t = cst[:, _OFF_BY:_OFF_BH]
            bh_t = cst[:, _OFF_BH:_OFF_BW]
            bw_t = cst[:, _OFF_BW:_OFF_FC1B]
            fc1b_t = cst[:, _OFF_FC1B:CONST_F]
            fc1T = zcst[:, _ZOFF_FC1:_ZOFF_S].rearrange("p (k o) -> p k o", k=KC)
            z2s_t = zcst[:, _ZOFF_S:_ZOFF_S + 1]
            z2b_t = zcst[:, _ZOFF_B:_ZOFF_B + 1]

            # pre-warm the sigmoid table set off the critical path
            warm = consts.tile([P, 1], F32)
            nc.scalar.activation(warm[:], cst[:, 0:1], AF.Sigmoid)

            NH = 2                 # phase-2 h-halves per chunk
            HH = H // NH           # 32

            # tiles whose z-add runs on DVE (4x ts-add) instead of Scalar:
            # the tail of the per-batch Scalar chain, so the last applies
            # are gated by sigmoid only and DVE never waits at batch
            # boundaries.  The final batch puts all six on DVE so Scalar
            # drains early (shrinks the pipeline tail).
            Z_DVE_STEADY = set()

            def phase2_mults(st, z_dve=False):
                # t = sigmoid(xh outer xw) (+ z unless deferred to DVE)
                x_sb, xh2, xw, z3f, b = st
                tiles = []
                for oc in range(KC):
                    for hh in range(NH):
                        h0 = hh * HH
                        i = oc * NH + hh
                        t_t = tpool.tile([P, HH, W], BF16, tag="t",
                                         name="t_t")
                        tq = t_t[:].rearrange("p h (a b) -> p h a b", b=2)
                        # xh pairs broadcast on middle dim -> 2x mode
                        nc.vector.tensor_mul(
                            tq,
                            xh2[:, oc, h0:h0 + HH, :].unsqueeze(2)
                               .broadcast_to([P, HH, W // 2, 2]),
                            xw[:, oc, :].rearrange("p (a b) -> p a b", b=2)
                               .unsqueeze(1)
                               .broadcast_to([P, HH, W // 2, 2]))
                        nc.scalar.activation(t_t[:], t_t[:], AF.Sigmoid)
                        if not (z_dve or i in Z_DVE_STEADY):
                            nc.scalar.activation(
                                t_t[:], t_t[:], AF.Identity,
                                bias=z3f[:, oc:oc + 1])
                        tiles.append(t_t)
                return tiles

            def phase2_applies(st, tiles, z_dve=False):
                # out = t * x in place over x, store per channel chunk
                x_sb, xh2, xw, z3f, b = st
                for oc in range(KC):
                    for hh in range(NH):
                        h0 = hh * HH
                        i = oc * NH + hh
                        t_t = tiles[i]
                        if z_dve or i in Z_DVE_STEADY:
                            nc.vector.tensor_scalar_add(
                                t_t[:], t_t[:], z3f[:, oc:oc + 1])
                        nc.vector.tensor_mul(
                            x_sb[:, oc, h0:h0 + HH, :],
                            x_sb[:, oc, h0:h0 + HH, :], t_t[:])
                    nc.sync.dma_start(
                        out_d.ap()[b, oc * P:(oc + 1) * P],
                        x_sb[:, oc, :, :])

            prev = None
            prev_tiles = None
            for b in range(BS):
                x_sb = xpool.tile([P, KC, H, W], BF16, tag="x", name="xsb")
                xs = x_d.ap()[b].rearrange("(k p) h w -> p k h w", p=P)
                hf = hfpool.tile([P, KC, 32, W], BF16, tag="hf")
                wt = wtpool.tile([P, KC, H, 32], BF16, tag="wt")
                y_pre = ypre_pool.tile([P, KC, H + W], BF16, tag="ypre")
                for kc in range(KC):
                    nc.sync.dma_start(x_sb[:, kc, :, :], xs[:, kc, :, :])

                for kc in range(KC):
                    # h-fold level 1: 64 -> 32 rows (contiguous, 2x mode)
                    nc.vector.tensor_add(
                        hf[:, kc, :, :], x_sb[:, kc, 0:32, :],
                        x_sb[:, kc, 32:64, :])
                    # w-fold level 1: 64 -> 32 cols (2x mode)
                    nc.vector.tensor_add(
                        wt[:, kc, :, :], x_sb[:, kc, :, 0:32],
                        x_sb[:, kc, :, 32:64])

                if prev is not None:
                    # the last phase-2 puts the z-add on DVE so Scalar
                    # drains early (shrinks the pipeline tail)
                    z_dve = b == BS - 1
                    prev_tiles = phase2_mults(prev, z_dve=z_dve)

                # w-tree levels 2..6 (in place), final lands in y_pre
                nc.vector.tensor_add(
                    wt[:, :, :, 0:16], wt[:, :, :, 0:16], wt[:, :, :, 16:32])
                nc.vector.tensor_add(
                    wt[:, :, :, 0:8], wt[:, :, :, 0:8], wt[:, :, :, 8:16])
                nc.vector.tensor_add(
                    wt[:, :, :, 0:4], wt[:, :, :, 0:4], wt[:, :, :, 4:8])
                nc.vector.tensor_add(
                    wt[:, :, :, 0:2], wt[:, :, :, 0:2], wt[:, :, :, 2:4])
                nc.vector.tensor_add(
                    y_pre[:, :, 0:H], wt[:, :, :, 0], wt[:, :, :, 1])
                # h-ladder 32 -> 1 (in place on hf), final lands in y_pre
                nc.vector.tensor_add(
                    hf[:, :, 0:16, :], hf[:, :, 0:16, :], hf[:, :, 16:32, :])
                nc.vector.tensor_add(
                    hf[:, :, 0:8, :], hf[:, :, 0:8, :], hf[:, :, 8:16, :])
                nc.vector.tensor_add(
                    hf[:, :, 0:4, :], hf[:, :, 0:4, :], hf[:, :, 4:8, :])
                nc.vector.tensor_add(
                    hf[:, :, 0:2, :], hf[:, :, 0:2, :], hf[:, :, 2:4, :])
                nc.vector.tensor_add(
                    y_pre[:, :, H:H + W], hf[:, :, 0, :], hf[:, :, 1, :])

                if prev is not None:
                    phase2_applies(prev, prev_tiles, z_dve=b == BS - 1)
                    prev = None

                # y = relu(Wy' @ y_pre + by), zraw = row sums of y
                psum_y = psy.tile([P, KC, H + W], F32, tag="py")
                for oc in range(KC):
                    for kc in range(KC):
                        nc.tensor.matmul(
                            psum_y[:, oc, :],
                            wyT[:, kc, oc * P:(oc + 1) * P],
                            y_pre[:, kc, :],
                            start=(kc == 0), stop=(kc == KC - 1))
                y_sb = ysb_pool.tile([P, KC, H + W], BF16, tag="y")
                zraw = zpool.tile([P, KC, 1], F32, tag="zraw")
                for oc in range(KC):
                    nc.scalar.activation(
                        y_sb[:, oc, :], psum_y[:, oc, :], AF.Relu,
                        bias=by_t[:, oc:oc + 1],
                        accum_out=zraw[:, oc, :])

                # z chain (fp32, tiny)
                psum_z = psz.tile([R, 1], F32, tag="pz")
                for kc in range(KC):
                    nc.tensor.matmul(
                        psum_z[:], fc0T[:, kc, :], zraw[:, kc, :],
                        start=(kc == 0), stop=(kc == KC - 1))
                z2 = zpool.tile([R, 1], F32, tag="z2")
                nc.scalar.activation(z2[:], psum_z[:], AF.Relu,
                                     bias=z2b_t[:], scale=z2s_t[:])
                psum_z3 = psz.tile([P, KC], F32, tag="pz3")
                for oc in range(KC):
                    nc.tensor.matmul(
                        psum_z3[:, oc:oc + 1], fc1T[:, oc, :], z2[:],
                        start=True, stop=True)
                z3f = zpool.tile([P, KC], F32, tag="z3")
                nc.vector.tensor_add(z3f[:], psum_z3[:], fc1b_t[:])

                # xh (duplicated pairs) and xw
                psum_hw = pshw.tile([P, KC, 3 * H], F32, tag="phw")
                for oc in range(KC):
                    for kc in range(KC):
                        nc.tensor.matmul(
                            psum_hw[:, oc, 0:2 * H],
                            whT[:, kc, oc * P:(oc + 1) * P],
                            y_sb[:, kc, 0:H].unsqueeze(2)
                                .broadcast_to([P, H, 2]),
                            start=(kc == 0), stop=(kc == KC - 1))
                    for kc in range(KC):
                        nc.tensor.matmul(
                            psum_hw[:, oc, 2 * H:3 * H],
                            wwT[:, kc, oc * P:(oc + 1) * P],
                            y_sb[:, kc, H:H + W],
                            start=(kc == 0), stop=(kc == KC - 1))
                xh2 = hw_pool.tile([P, KC, H, 2], BF16, tag="xh2")
                xw = hw_pool.tile([P, KC, W], BF16, tag="xw")
                for oc in range(KC):
                    nc.scalar.activation(
                        xh2[:, oc, :, :], psum_hw[:, oc, 0:2 * H], AF.Relu,
                        bias=bh_t[:, oc:oc + 1])
                    nc.scalar.activation(
                        xw[:, oc, :], psum_hw[:, oc, 2 * H:3 * H], AF.Relu,
                        bias=bw_t[:, oc:oc + 1])

                prev = (x_sb, xh2, xw, z3f, b)
            prev_tiles = phase2_mults(prev, z_dve=True)
            phase2_applies(prev, prev_tiles, z_dve=True)

    nc.compile()
    return nc


def _pack_consts(Wy, gy, by, Wh, gh, bh, Ww, gw, bw,
                 fc0_w, fc0_b, bn1_g, bn1_b, fc1_w, fc1_b):
    inv = 1.0 / np.sqrt(1.0 + EPS)

    def chunked_T(w):
        # [out, in] -> lhsT tile [p, kc, out]
        return np.ascontiguousarray(
            w.T.reshape(KC, P, C).transpose(1, 0, 2))

    def lanes(v):
        # [C] -> [p, kc]
        return np.ascontiguousarray(v.reshape(KC, P).T)

    wbl = np.empty((P, WBLOB_F), np.float32)
    wbl[:, _OFF_WY:_OFF_WH] = chunked_T(
        Wy * (gy * inv)[:, None] / 64.0).reshape(P, KC * C)
    wbl[:, _OFF_WH:_OFF_WW] = chunked_T(
        Wh * (gh * inv)[:, None]).reshape(P, KC * C)
    wbl[:, _OFF_WW:WBLOB_F] = chunked_T(
        Ww * (gw * inv)[:, None]).reshape(P, KC * C)
    cst = np.empty((P, CONST_F), np.float32)
    # wavelet level-i scale per channel chunk, folded into fc0
    wscale = np.repeat(2.0 ** (np.arange(1, KC + 1) / 2.0) / (H + W), P)
    fc0T_s = (fc0_w * wscale[None, :]).T        # [C, R]
    cst[:, _OFF_FC0:_OFF_BY] = fc0T_s.reshape(KC, P, R).transpose(1, 0, 2) \
                                     .reshape(P, KC * R)
    cst[:, _OFF_BY:_OFF_BH] = lanes(by)
    cst[:, _OFF_BH:_OFF_BW] = lanes(bh)
    cst[:, _OFF_BW:_OFF_FC1B] = lanes(bw)
    cst[:, _OFF_FC1B:CONST_F] = lanes(fc1_b)

    zcst = np.empty((R, ZCONST_F), np.float32)
    zcst[:, _ZOFF_FC1:_ZOFF_S] = fc1_w.T.reshape(R, KC * P)
    z2s = bn1_g * inv
    zcst[:, _ZOFF_S] = z2s
    zcst[:, _ZOFF_B] = fc0_b * z2s + bn1_b
    return wbl.astype(BF), cst, zcst


def _get_compiled():
    global _compiled
    if _compiled is None:
        _compiled = _build()
    return _compiled


def kernel(x, Wy, gy, by, Wh, gh, bh, Ww, gw, bw,
           fc0_w, fc0_b, bn1_g, bn1_b, fc1_w, fc1_b,
           _trace=False, _trace_kwargs=None):
    nc = _get_compiled()
    wbl, cst, zcst = _pack_consts(
        np.asarray(Wy, np.float32), np.asarray(gy, np.float32),
        np.asarray(by, np.float32), np.asarray(Wh, np.float32),
        np.asarray(gh, np.float32), np.asarray(bh, np.float32),
        np.asarray(Ww, np.float32), np.asarray(gw, np.float32),
        np.asarray(bw, np.float32), np.asarray(fc0_w, np.float32),
        np.asarray(fc0_b, np.float32), np.asarray(bn1_g, np.float32),
        np.asarray(bn1_b, np.float32), np.asarray(fc1_w, np.float32),
        np.asarray(fc1_b, np.float32))
    x = np.ascontiguousarray(np.asarray(x, np.float32)).astype(BF)
    in_maps = [
        {"x": x[i * BS:(i + 1) * BS], "wbl": wbl, "cst": cst, "zcst": zcst}
        for i in range(N_CORES)
    ]
    res = run_bass_kernel_spmd(
        nc, in_maps, list(range(N_CORES)),
        trace=_trace, **(_trace_kwargs or {}))
    out = np.concatenate(
        [np.asarray(res.results[i]["out"]).astype(np.float32)
         for i in range(N_CORES)], axis=0)
    if _trace:
        return out, res
    return out


# revision 27
# speedup vs baseline: 1.0168x; 1.0168x over previous
"""Trainium2 Bass kernel for the CoordPooling+SFP gate module (bf16).

Computation (per batch b):
  y_pre = [sum_w x | sum_h x]                 [C, H+W]   (C=384, H=W=64)
  y  = relu((sy*Wy/64 @ y_pre) + by)          [C, 128]
  xh = relu((sh*Wh @ y[:, :64]) + bh)         [C, 64]
  xw = relu((sw*Ww @ y[:, 64:]) + bw)         [C, 64]
  z_raw[c] = sum_L y[c, :]
  z  = fc1(relu(bn1(fc0(z_raw * wscale))))    [C]
  out = x * (sigmoid(xh outer xw) + z)

Everything on-chip is bf16 (HBM traffic halved; DVE 2x mode on packed
ops); accumulations happen in fp32 inside the engines.  Reductions are
TT fold trees (tensor_reduce gets no bf16 speedup).  The outer product
runs at DVE 2x via duplicated xh pairs: the hw-matmul streams each y
column twice so relu lands xh as [h,2] pairs, letting the broadcast sit
on a middle AP dim with a stride-1 innermost pair.

Sharding: data-parallel over batch, 4 batches per core on 8 cores.
"""

import sys
import numpy as np
import ml_dtypes

for _p in ("/opt/trn_rl_repo", "/root/.axon_site/_ro/trn_rl_repo"):
    if _p not in sys.path:
        sys.path.append(_p)

def _ensure_ntff_hook():
    # bass_utils imports antenv.axon_hooks when tracing is requested
    # (e.g. BASS_TRACE=1); this image's antenv lacks that module, which
    # would crash the run.  Register a functional shim (real ctypes hook
    # when available, else a None-hook so tracing degrades gracefully).
    try:
        import antenv.axon_hooks  # noqa: F401
        return
    except ImportError:
        pass
    import types
    try:
        import antenv
    except ImportError:
        return
    mod = types.ModuleType("antenv.axon_hooks")
    _hook = [None]
    mod.set_axon_ntff_profile_hook = lambda h: _hook.__setitem__(0, h)
    mod.get_axon_ntff_profile_hook = lambda: _hook[0]
    sys.modules["antenv.axon_hooks"] = mod
    antenv.axon_hooks = mod
    try:
        from trn_agent_boot.trn_boot import _ntff_profile_via_ctypes
        mod.set_axon_ntff_profile_hook(
            _ntff_profile_via_ctypes("/opt/axon/libaxon_pjrt.so"))
    except Exception:
        pass


_ensure_ntff_hook()

import concourse.bass as bass
import concourse.tile as tile
from concourse import bacc, mybir
from concourse.bass_utils import run_bass_kernel_spmd

F32 = mybir.dt.float32
BF16 = mybir.dt.bfloat16
AF = mybir.ActivationFunctionType
OP = mybir.AluOpType
BF = ml_dtypes.bfloat16

N_CORES = 8
B, C, H, W = 32, 384, 64, 64
BS = B // N_CORES          # batches per core
P = 128                    # partitions
KC = C // P                # channel chunks (3)
R = 24                     # gate bottleneck
EPS = 1e-5

# const blob layout (free-dim offsets within [128, CONST_F])
_OFF_WY = 0
_OFF_WH = _OFF_WY + KC * C      # 1152
_OFF_WW = _OFF_WH + KC * C
WBLOB_F = _OFF_WW + KC * C      # 3456 (bf16 blob)
_OFF_FC0 = 0
_OFF_BY = _OFF_FC0 + KC * R     # 72
_OFF_BH = _OFF_BY + KC
_OFF_BW = _OFF_BH + KC
_OFF_FC1B = _OFF_BW + KC
CONST_F = _OFF_FC1B + KC        # 84

_ZOFF_FC1 = 0
_ZOFF_S = KC * P                # 384
_ZOFF_B = _ZOFF_S + 1
ZCONST_F = _ZOFF_B + 1          # 386

_compiled = None


def _build():
    nc = bacc.Bacc("TRN2", target_bir_lowering=False, debug=False,
                   num_devices=N_CORES)
    x_d = nc.dram_tensor("x", [BS, C, H, W], BF16, kind="ExternalInput")
    wbl_d = nc.dram_tensor("wbl", [P, WBLOB_F], BF16, kind="ExternalInput")
    cst_d = nc.dram_tensor("cst", [P, CONST_F], F32, kind="ExternalInput")
    zcst_d = nc.dram_tensor("zcst", [R, ZCONST_F], F32, kind="ExternalInput")
    out_d = nc.dram_tensor("out", [BS, C, H, W], BF16, kind="ExternalOutput")

    with tile.TileContext(nc) as tc:
        with (
            nc.allow_low_precision("bf16 pipeline, tolerance 2e-2"),
            tc.tile_pool(name="consts", bufs=1) as consts,
            tc.tile_pool(name="xp", bufs=4) as xpool,
            tc.tile_pool(name="wtp", bufs=2) as wtpool,
            tc.tile_pool(name="hfp", bufs=2) as hfpool,
            tc.tile_pool(name="ypre", bufs=2) as ypre_pool,
            tc.tile_pool(name="ysb", bufs=2) as ysb_pool,
            tc.tile_pool(name="hwp", bufs=2) as hw_pool,
            tc.tile_pool(name="zp", bufs=2) as zpool,
            tc.tile_pool(name="tp", bufs=8) as tpool,
            tc.tile_pool(name="psy", bufs=2, space=bass.MemorySpace.PSUM) as psy,
            tc.tile_pool(name="pshw", bufs=2, space=bass.MemorySpace.PSUM) as pshw,
            tc.tile_pool(name="psz", bufs=1, space=bass.MemorySpace.PSUM) as psz,
        ):
            wbl = consts.tile([P, WBLOB_F], BF16)
            nc.scalar.dma_start(wbl[:], wbl_d.ap())
            cst = consts.tile([P, CONST_F], F32)
            nc.scalar.dma_start(cst[:], cst_d.ap())
            zcst = consts.tile([R, ZCONST_F], F32)
            nc.scalar.dma_start(zcst[:], zcst_d.ap())

            wyT = wbl[:, _OFF_WY:_OFF_WH].rearrange("p (k o) -> p k o", k=KC)
            whT = wbl[:, _OFF_WH:_OFF_WW].rearrange("p (k o) -> p k o", k=KC)
            wwT = wbl[:, _OFF_WW:WBLOB_F].rearrange("p (k o) -> p k o", k=KC)
            fc0T = cst[:, _OFF_FC0:_OFF_BY].rearrange("p (k r) -> p k r", k=KC)
            by_t = cst[:, _OFF_BY:_OFF_BH]
            bh_t = cst[:, _OFF_BH:_OFF_BW]
            bw_t = cst[:, _OFF_BW:_OFF_FC1B]
            fc1b_t = cst[:, _OFF_FC1B:CONST_F]
            fc1T = zcst[:, _ZOFF_FC1:_ZOFF_S].rearrange("p (k o) -> p k o", k=KC)
            z2s_t = zcst[:, _ZOFF_S:_ZOFF_S + 1]
            z2b_t = zcst[:, _ZOFF_B:_ZOFF_B + 1]

            # pre-warm the sigmoid table set off the critical path
            warm = consts.tile([P, 1], F32)
            nc.scalar.activation(warm[:], cst[:, 0:1], AF.Sigmoid)

            NH = 2                 # phase-2 h-halves per chunk
            HH = H // NH           # 32

            # tiles whose z-add runs on DVE (4x ts-add) instead of Scalar:
            # the tail of the per-batch Scalar chain, so the last applies
            # are gated by sigmoid only and DVE never waits at batch
            # boundaries.  The final batch puts all six on DVE so Scalar
            # drains early (shrinks the pipeline tail).
            Z_DVE_STEADY = set()

            def phase2_mults(st, z_dve=False):
                # t = sigmoid(xh outer xw) (+ z unless deferred to DVE)
                x_sb, xh2, xw, z3f, b = st
                tiles = []
                for oc in range(KC):
                    for hh in range(NH):
                        h0 = hh * HH
                        i = oc * NH + hh
                        t_t = tpool.tile([P, HH, W], BF16, tag="t",
                                         name="t_t")
                        tq = t_t[:].rearrange("p h (a b) -> p h a b", b=2)
                        # xh pairs broadcast on middle dim -> 2x mode
                        nc.vector.tensor_mul(
                            tq,
                            xh2[:, oc, h0:h0 + HH, :].unsqueeze(2)
                               .broadcast_to([P, HH, W // 2, 2]),
                            xw[:, oc, :].rearrange("p (a b) -> p a b", b=2)
                               .unsqueeze(1)
                               .broadcast_to([P, HH, W // 2, 2]))
                        nc.scalar.activation(t_t[:], t_t[:], AF.Sigmoid)
                        if not (z_dve or i in Z_DVE_STEADY):
                            nc.scalar.activation(
                                t_t[:], t_t[:], AF.Identity,
                                bias=z3f[:, oc:oc + 1])
                        tiles.append(t_t)
                return tiles

            def phase2_applies(st, tiles, z_dve=False):
                # out = t * x in place over x, store per channel chunk
                x_sb, xh2, xw, z3f, b = st
                for oc in range(KC):
                    for hh in range(NH):
                        h0 = hh * HH
                        i = oc * NH + hh
                        t_t = tiles[i]
                        if z_dve or i in Z_DVE_STEADY:
                            nc.vector.tensor_scalar_add(
                                t_t[:], t_t[:], z3f[:, oc:oc + 1])
                        nc.vector.tensor_mul(
                            x_sb[:, oc, h0:h0 + HH, :],
                            x_sb[:, oc, h0:h0 + HH, :], t_t[:])
                        if z_dve:
                            # final batch: drain each half immediately
                            nc.sync.dma_start(
                                out_d.ap()[b, oc * P:(oc + 1) * P,
                                           h0:h0 + HH],
                                x_sb[:, oc, h0:h0 + HH, :])
                    if not z_dve:
                        nc.sync.dma_start(
                            out_d.ap()[b, oc * P:(oc + 1) * P],
                            x_sb[:, oc, :, :])

            prev = None
            prev_tiles = None
            for b in range(BS):
                x_sb = xpool.tile([P, KC, H, W], BF16, tag="x", name="xsb")
                xs = x_d.ap()[b].rearrange("(k p) h w -> p k h w", p=P)
                hf = hfpool.tile([P, KC, 32, W], BF16, tag="hf")
                wt = wtpool.tile([P, KC, H, 32], BF16, tag="wt")
                y_pre = ypre_pool.tile([P, KC, H + W], BF16, tag="ypre")
                if b == 0:
                    # first batch: load h-halves so folding starts while
                    # the rest of the prefetch is still in flight
                    for kc in range(KC):
                        for g in range(2):
                            nc.sync.dma_start(
                                x_sb[:, kc, 32 * g:32 * (g + 1), :],
                                xs[:, kc, 32 * g:32 * (g + 1), :])
                else:
                    for kc in range(KC):
                        nc.sync.dma_start(x_sb[:, kc, :, :],
                                          xs[:, kc, :, :])

                for kc in range(KC):
                    if b == 0:
                        # w-fold per h-half: each needs only one sub-DMA
                        for g in range(2):
                            nc.vector.tensor_add(
                                wt[:, kc, 32 * g:32 * (g + 1), :],
                                x_sb[:, kc, 32 * g:32 * (g + 1), 0:32],
                                x_sb[:, kc, 32 * g:32 * (g + 1), 32:64])
                        nc.vector.tensor_add(
                            hf[:, kc, :, :], x_sb[:, kc, 0:32, :],
                            x_sb[:, kc, 32:64, :])
                        continue
                    # h-fold level 1: 64 -> 32 rows (contiguous, 2x mode)
                    nc.vector.tensor_add(
                        hf[:, kc, :, :], x_sb[:, kc, 0:32, :],
                        x_sb[:, kc, 32:64, :])
                    # w-fold level 1: 64 -> 32 cols (2x mode)
                    nc.vector.tensor_add(
                        wt[:, kc, :, :], x_sb[:, kc, :, 0:32],
                        x_sb[:, kc, :, 32:64])

                if prev is not None:
                    # the last phase-2 puts the z-add on DVE so Scalar
                    # drains early (shrinks the pipeline tail)
                    z_dve = b == BS - 1
                    prev_tiles = phase2_mults(prev, z_dve=z_dve)

                # w-tree levels 2..6 (in place), final lands in y_pre
                nc.vector.tensor_add(
                    wt[:, :, :, 0:16], wt[:, :, :, 0:16], wt[:, :, :, 16:32])
                nc.vector.tensor_add(
                    wt[:, :, :, 0:8], wt[:, :, :, 0:8], wt[:, :, :, 8:16])
                nc.vector.tensor_add(
                    wt[:, :, :, 0:4], wt[:, :, :, 0:4], wt[:, :, :, 4:8])
                nc.vector.tensor_add(
                    wt[:, :, :, 0:2], wt[:, :, :, 0:2], wt[:, :, :, 2:4])
                nc.vector.tensor_add(
                    y_pre[:, :, 0:H], wt[:, :, :, 0], wt[:, :, :, 1])
                # h-ladder 32 -> 1 (in place on hf), final lands in y_pre
                nc.vector.tensor_add(
                    hf[:, :, 0:16, :], hf[:, :, 0:16, :], hf[:, :, 16:32, :])
                nc.vector.tensor_add(
                    hf[:, :, 0:8, :], hf[:, :, 0:8, :], hf[:, :, 8:16, :])
                nc.vector.tensor_add(
                    hf[:, :, 0:4, :], hf[:, :, 0:4, :], hf[:, :, 4:8, :])
                nc.vector.tensor_add(
                    hf[:, :, 0:2, :], hf[:, :, 0:2, :], hf[:, :, 2:4, :])
                nc.vector.tensor_add(
                    y_pre[:, :, H:H + W], hf[:, :, 0, :], hf[:, :, 1, :])

                if prev is not None:
                    phase2_applies(prev, prev_tiles, z_dve=b == BS - 1)
                    prev = None

                # y = relu(Wy' @ y_pre + by), zraw = row sums of y
                psum_y = psy.tile([P, KC, H + W], F32, tag="py")
                for oc in range(KC):
                    for kc in range(KC):
                        nc.tensor.matmul(
                            psum_y[:, oc, :],
                            wyT[:, kc, oc * P:(oc + 1) * P],
                            y_pre[:, kc, :],
                            start=(kc == 0), stop=(kc == KC - 1))
                y_sb = ysb_pool.tile([P, KC, H + W], BF16, tag="y")
                zraw = zpool.tile([P, KC, 1], F32, tag="zraw")
                for oc in range(KC):
                    nc.scalar.activation(
                        y_sb[:, oc, :], psum_y[:, oc, :], AF.Relu,
                        bias=by_t[:, oc:oc + 1],
                        accum_out=zraw[:, oc, :])

                # z chain (fp32, tiny)
                psum_z = psz.tile([R, 1], F32, tag="pz")
                for kc in range(KC):
                    nc.tensor.matmul(
                        psum_z[:], fc0T[:, kc, :], zraw[:, kc, :],
                        start=(kc == 0), stop=(kc == KC - 1))
                z2 = zpool.tile([R, 1], F32, tag="z2")
                nc.scalar.activation(z2[:], psum_z[:], AF.Relu,
                                     bias=z2b_t[:], scale=z2s_t[:])
                psum_z3 = psz.tile([P, KC], F32, tag="pz3")
                for oc in range(KC):
                    nc.tensor.matmul(
                        psum_z3[:, oc:oc + 1], fc1T[:, oc, :], z2[:],
                        start=True, stop=True)
                z3f = zpool.tile([P, KC], F32, tag="z3")
                nc.vector.tensor_add(z3f[:], psum_z3[:], fc1b_t[:])

                # xh (duplicated pairs) and xw
                psum_hw = pshw.tile([P, KC, 3 * H], F32, tag="phw")
                for oc in range(KC):
                    for kc in range(KC):
                        nc.tensor.matmul(
                            psum_hw[:, oc, 0:2 * H],
                            whT[:, kc, oc * P:(oc + 1) * P],
                            y_sb[:, kc, 0:H].unsqueeze(2)
                                .broadcast_to([P, H, 2]),
                            start=(kc == 0), stop=(kc == KC - 1))
                    for kc in range(KC):
                        nc.tensor.matmul(
                            psum_hw[:, oc, 2 * H:3 * H],
                            wwT[:, kc, oc * P:(oc + 1) * P],
                            y_sb[:, kc, H:H + W],
                            start=(kc == 0), stop=(kc == KC - 1))
                xh2 = hw_pool.tile([P, KC, H, 2], BF16, tag="xh2")
                xw = hw_pool.tile([P, KC, W], BF16, tag="xw")
                for oc in range(KC):
                    nc.scalar.activation(
                        xh2[:, oc, :, :], psum_hw[:, oc, 0:2 * H], AF.Relu,
                        bias=bh_t[:, oc:oc + 1])
                    nc.scalar.activation(
                        xw[:, oc, :], psum_hw[:, oc, 2 * H:3 * H], AF.Relu,
                        bias=bw_t[:, oc:oc + 1])

                prev = (x_sb, xh2, xw, z3f, b)
            prev_tiles = phase2_mults(prev, z_dve=True)
            phase2_applies(prev, prev_tiles, z_dve=True)

    nc.compile()
    return nc


def _pack_consts(Wy, gy, by, Wh, gh, bh, Ww, gw, bw,
                 fc0_w, fc0_b, bn1_g, bn1_b, fc1_w, fc1_b):
    inv = 1.0 / np.sqrt(1.0 + EPS)

    def chunked_T(w):
        # [out, in] -> lhsT tile [p, kc, out]
        return np.ascontiguousarray(
            w.T.reshape(KC, P, C).transpose(1, 0, 2))

    def lanes(v):
        # [C] -> [p, kc]
        return np.ascontiguousarray(v.reshape(KC, P).T)

    wbl = np.empty((P, WBLOB_F), np.float32)
    wbl[:, _OFF_WY:_OFF_WH] = chunked_T(
        Wy * (gy * inv)[:, None] / 64.0).reshape(P, KC * C)
    wbl[:, _OFF_WH:_OFF_WW] = chunked_T(
        Wh * (gh * inv)[:, None]).reshape(P, KC * C)
    wbl[:, _OFF_WW:WBLOB_F] = chunked_T(
        Ww * (gw * inv)[:, None]).reshape(P, KC * C)
    cst = np.empty((P, CONST_F), np.float32)
    # wavelet level-i scale per channel chunk, folded into fc0
    wscale = np.repeat(2.0 ** (np.arange(1, KC + 1) / 2.0) / (H + W), P)
    fc0T_s = (fc0_w * wscale[None, :]).T        # [C, R]
    cst[:, _OFF_FC0:_OFF_BY] = fc0T_s.reshape(KC, P, R).transpose(1, 0, 2) \
                                     .reshape(P, KC * R)
    cst[:, _OFF_BY:_OFF_BH] = lanes(by)
    cst[:, _OFF_BH:_OFF_BW] = lanes(bh)
    cst[:, _OFF_BW:_OFF_FC1B] = lanes(bw)
    cst[:, _OFF_FC1B:CONST_F] = lanes(fc1_b)

    zcst = np.empty((R, ZCONST_F), np.float32)
    zcst[:, _ZOFF_FC1:_ZOFF_S] = fc1_w.T.reshape(R, KC * P)
    z2s = bn1_g * inv
    zcst[:, _ZOFF_S] = z2s
    zcst[:, _ZOFF_B] = fc0_b * z2s + bn1_b
    return wbl.astype(BF), cst, zcst


def _get_compiled():
    global _compiled
    if _compiled is None:
        _compiled = _build()
    return _compiled


def kernel(x, Wy, gy, by, Wh, gh, bh, Ww, gw, bw,
           fc0_w, fc0_b, bn1_g, bn1_b, fc1_w, fc1_b,
           _trace=False, _trace_kwargs=None):
    nc = _get_compiled()
    wbl, cst, zcst = _pack_consts(
        np.asarray(Wy, np.float32), np.asarray(gy, np.float32),
        np.asarray(by, np.float32), np.asarray(Wh, np.float32),
        np.asarray(gh, np.float32), np.asarray(bh, np.float32),
        np.asarray(Ww, np.float32), np.asarray(gw, np.float32),
        np.asarray(bw, np.float32), np.asarray(fc0_w, np.float32),
        np.asarray(fc0_b, np.float32), np.asarray(bn1_g, np.float32),
        np.asarray(bn1_b, np.float32), np.asarray(fc1_w, np.float32),
        np.asarray(fc1_b, np.float32))
    x = np.ascontiguousarray(np.asarray(x, np.float32)).astype(BF)
    in_maps = [
        {"x": x[i * BS:(i + 1) * BS], "wbl": wbl, "cst": cst, "zcst": zcst}
        for i in range(N_CORES)
    ]
    res = run_bass_kernel_spmd(
        nc, in_maps, list(range(N_CORES)),
        trace=_trace, **(_trace_kwargs or {}))
    out = np.concatenate(
        [np.asarray(res.results[i]["out"]).astype(np.float32)
         for i in range(N_CORES)], axis=0)
    if _trace:
        return out, res
    return out
